# revision 9
# baseline (speedup 1.0000x reference)
"""Trainium2 Bass kernel for nn_DSA (dual-stage attention RNN).

Mathematical collapse used (exact, not approximate):
  - In the reference scan, beta = log_softmax(sc, axis=-1) over a SIZE-1
    axis, which is identically zero for any finite input.  Hence
    ctx_new = einsum('bt,bth->bh', 0, enc_h) == 0 exactly, so the carried
    context is zero at every step and the decoder input at step t is
    din_t = d[:, t] * dec_w[0,0] + dec_b[0].
  - The carried h_s is never read inside the step, so only the final
    step's h_s (t = T-2) reaches the head.  The encoder LSTM, s1, and the
    whole attention pipeline are dead code w.r.t. the output.
  - feat = [h_s, ctx] with ctx == 0, so the head reduces to
      out[b] = h_s[b,:] @ v + k0,
      v  = d1_w[:, :H].T @ d2_w[0,:],     k0 = d1_b @ d2_w[0,:] + d2_b[0]
  where h_s = sigmoid(o) * tanh(sigmoid(i) * tanh(g)) and
  [i,f,g,o] = din * W_ih_d[:,0] + b_d  (f unused since c0 == 0).

Sharding: pure data parallel over batch (B=32 -> 4 rows per core x 8).
All weights replicated; each core computes its 4 outputs independently.
Host-side work is layout only (slicing / replication / concatenation);
every arithmetic op ((d*dw+db), the LSTM cell, v, k0, h@v+k0) runs on
device.

Device schedule (per core, BS=4, batch on partitions):
  - TWO input DMAs on separate queues (sync HWDGE + gpsimd SWDGE):
      packM (BS, 776): [W_i|W_o|W_g | b_i|b_o|b_g | d_col dw db d2b 1x4]
      packB (H, 133):  [d1_w[:, :H] | d2w_col xBS | d1b_col]
  - DVE: din = d*dw+db; z = Wrep*din + brep (split io/g so the sigmoid
    starts earlier); ACT: one Sigmoid on (BS,256) covers both gates
    (no DMA on the Activation queue, so its function table loads once).
  - PE (off critical path): vrep = (d2w x4).T @ d1w; krep accumulates
    d1b.d2w + d2b via a ones-row matmul (ones baked into packM).
  - finale: mul + reduce + scalar-add (tensor_tensor_reduce faults the
    exec unit on HW; stick to plain DVE ops).
"""

import numpy as np

import concourse.bacc as bacc
import concourse.bass as bass
import concourse.mybir as mybir
import concourse.tile as tile
from concourse import bass_utils

N_CORES = 8
B, T, H, L = 32, 100, 128, 64
BS = B // N_CORES  # batch rows per core

F32 = mybir.dt.float32
AF = mybir.ActivationFunctionType
ALU = mybir.AluOpType

PM_COLS = 6 * H + 8   # [W(384) | b(384) | d dw db d2b | 1 1 1 1]
PB_COLS = H + BS + 1  # [d1w (H) | d2w_col xBS | d1b_col]

_BUILD_CACHE = {}


def _build_nc():
    nc = bacc.Bacc("TRN2", target_bir_lowering=False, debug=False)

    packM = nc.dram_tensor("packM", (BS, PM_COLS), F32, kind="ExternalInput")
    packB = nc.dram_tensor("packB", (H, PB_COLS), F32, kind="ExternalInput")
    out = nc.dram_tensor("out", (BS, 1), F32, kind="ExternalOutput")

    W0, B0, X0 = 0, 3 * H, 6 * H  # pack section offsets

    with tile.TileContext(nc) as tc:
            pm = nc.alloc_sbuf_tensor("pm", [BS, PM_COLS], F32)
            pb = nc.alloc_sbuf_tensor("pb", [H, PB_COLS], F32)
            nc.sync.dma_start(pm[:, :], packM[:, :])
            nc.gpsimd.dma_start(pb[:, :], packB[:, :])

            # din = d * dec_w00 + dec_b0            (BS,1)
            din = nc.alloc_sbuf_tensor("din", [BS, 1], F32)
            nc.vector.tensor_scalar(
                din[:, :], pm[:, X0:X0 + 1],
                pm[:, X0 + 1:X0 + 2], pm[:, X0 + 2:X0 + 3],
                ALU.mult, ALU.add,
            )
            # z = Wrep * din + brep, gates [i|o|g]; io first so ACT starts early
            z = nc.alloc_sbuf_tensor("z", [BS, 3 * H], F32)
            nc.vector.scalar_tensor_tensor(
                z[:, 0:2 * H], pm[:, W0:W0 + 2 * H], din[:, :],
                pm[:, B0:B0 + 2 * H], ALU.mult, ALU.add,
            )
            nc.vector.scalar_tensor_tensor(
                z[:, 2 * H:3 * H], pm[:, W0 + 2 * H:W0 + 3 * H], din[:, :],
                pm[:, B0 + 2 * H:B0 + 3 * H], ALU.mult, ALU.add,
            )

            sio = nc.alloc_sbuf_tensor("sio", [BS, 2 * H], F32)
            tg = nc.alloc_sbuf_tensor("tg", [BS, H], F32)
            nc.scalar.activation(sio[:, :], z[:, 0:2 * H], AF.Sigmoid)
            nc.scalar.activation(tg[:, :], z[:, 2 * H:3 * H], AF.Tanh)
            cst = nc.alloc_sbuf_tensor("cst", [BS, H], F32)
            nc.vector.tensor_mul(cst[:, :], sio[:, 0:H], tg[:, :])
            tcs = nc.alloc_sbuf_tensor("tcs", [BS, H], F32)
            nc.scalar.activation(tcs[:, :], cst[:, :], AF.Tanh)
            hst = nc.alloc_sbuf_tensor("hst", [BS, H], F32)
            nc.vector.tensor_mul(hst[:, :], sio[:, H:2 * H], tcs[:, :])

            # vrep[b,h] = sum_j d2w[j] * d1w[j,h]   (BS, H)
            vrep = nc.alloc_psum_tensor("vrep", [BS, H])
            nc.tensor.matmul(
                vrep[:, :], pb[:, H:H + BS], pb[:, 0:H], start=True, stop=True
            )
            # krep[b] = sum_j d2w[j] * d1b[j] + d2b (BS, 1)
            krep = nc.alloc_psum_tensor("krep", [BS, 1])
            nc.tensor.matmul(
                krep[:, :], pb[:, H:H + BS], pb[:, H + BS:H + BS + 1],
                start=True, stop=False,
            )
            nc.tensor.matmul(
                krep[:, :], pm[0:1, X0 + 4:X0 + 8], pm[0:1, X0 + 3:X0 + 4],
                start=False, stop=True,
            )

            # out[b] = sum_h h[b,h]*vrep[b,h] + krep[b]: krep is copied
            # (early, off the critical path) into an extra scratch column
            # so one widened reduce covers the +k0 as well
            scratch = nc.alloc_sbuf_tensor("scratch", [BS, H + 1], F32)
            res = nc.alloc_sbuf_tensor("res", [BS, 1], F32)
            nc.vector.tensor_copy(scratch[:, H:H + 1], krep[:, 0:1])
            nc.vector.tensor_mul(scratch[:, 0:H], hst[:, :], vrep[:, :])
            nc.vector.tensor_reduce(
                res[:, :], scratch[:, :], mybir.AxisListType.X, ALU.add
            )
            nc.sync.dma_start(out[:, :], res[:, :])

    nc.compile()
    return nc


def get_nc():
    if "nc" not in _BUILD_CACHE:
        _BUILD_CACHE["nc"] = _build_nc()
    return _BUILD_CACHE["nc"]


def make_in_maps(inputs):
    f = lambda k: np.asarray(inputs[k], dtype=np.float32)
    d = f("d")
    wihd = f("W_ih_d").reshape(4 * H)
    b_d = f("b_d").reshape(4 * H)
    dw = f("dec_w").reshape(1, H + 1)[0, 0]
    db = f("dec_b").reshape(1)[0]
    d1w = f("d1_w").reshape(H, 2 * H)
    d1b = f("d1_b").reshape(H)
    d2w = f("d2_w").reshape(H)
    d2b = f("d2_b").reshape(1)[0]

    X0 = 6 * H
    base = np.empty(PM_COLS, np.float32)  # batch-independent part
    base[0:H] = wihd[0:H]                  # W_i
    base[H:2 * H] = wihd[3 * H:4 * H]      # W_o
    base[2 * H:3 * H] = wihd[2 * H:3 * H]  # W_g
    base[3 * H:4 * H] = b_d[0:H]
    base[4 * H:5 * H] = b_d[3 * H:4 * H]
    base[5 * H:6 * H] = b_d[2 * H:3 * H]
    base[X0 + 1] = dw
    base[X0 + 2] = db
    base[X0 + 3] = d2b
    base[X0 + 4:X0 + 8] = 1.0

    packB = np.empty((H, PB_COLS), np.float32)
    packB[:, 0:H] = d1w[:, 0:H]
    packB[:, H:H + BS] = d2w[:, None]
    packB[:, H + BS] = d1b

    in_maps = []
    for c in range(N_CORES):
        packM = np.tile(base, (BS, 1))
        packM[:, X0] = d[c * BS:(c + 1) * BS, T - 2]  # this core's d[:, T-2]
        in_maps.append({"packM": packM, "packB": packB})
    return in_maps


def run_spmd(inputs, trace=False):
    """Returns (full_output (B,), BassKernelResults)."""
    nc = get_nc()
    res = bass_utils.run_bass_kernel_spmd(
        nc, make_in_maps(inputs), list(range(N_CORES)), trace=trace
    )
    outs = [np.asarray(res.results[c]["out"]).reshape(BS) for c in range(N_CORES)]
    full = np.concatenate(outs).astype(np.float32)
    return full, res


def kernel(**inputs) -> np.ndarray:
    full, _ = run_spmd(inputs, trace=False)
    return full


# revision 10
# speedup vs baseline: 1.0054x; 1.0054x over previous
"""Trainium2 Bass kernel for nn_DSA (dual-stage attention RNN).

Mathematical collapse used (exact, not approximate):
  - In the reference scan, beta = log_softmax(sc, axis=-1) over a SIZE-1
    axis, which is identically zero for any finite input.  Hence
    ctx_new = einsum('bt,bth->bh', 0, enc_h) == 0 exactly, so the carried
    context is zero at every step and the decoder input at step t is
    din_t = d[:, t] * dec_w[0,0] + dec_b[0].
  - The carried h_s is never read inside the step, so only the final
    step's h_s (t = T-2) reaches the head.  The encoder LSTM, s1, and the
    whole attention pipeline are dead code w.r.t. the output.
  - feat = [h_s, ctx] with ctx == 0, so the head reduces to
      out[b] = h_s[b,:] @ v + k0,
      v  = d1_w[:, :H].T @ d2_w[0,:],     k0 = d1_b @ d2_w[0,:] + d2_b[0]
  where h_s = sigmoid(o) * tanh(sigmoid(i) * tanh(g)) and
  [i,f,g,o] = din * W_ih_d[:,0] + b_d  (f unused since c0 == 0).

Sharding: pure data parallel over batch (B=32 -> 4 rows per core x 8).
All weights replicated; each core computes its 4 outputs independently.
Host-side work is layout only (slicing / replication / concatenation);
every arithmetic op ((d*dw+db), the LSTM cell, v, k0, h@v+k0) runs on
device.

Device schedule (per core, BS=4, batch on partitions):
  - TWO input DMAs, both on the sync HWDGE queue (packM first - it
    feeds the critical DVE/ACT chain; packB only feeds PE, which has
    ~2us of slack; keeping gpsimd/scalar DMA-free avoids the flaky
    SWDGE drain and ACT table reloads):
      packM (BS, 776): [W_i|W_o|W_g | b_i|b_o|b_g | d_col dw db d2b 1x4]
      packB (H, 133):  [d1_w[:, :H] | d2w_col xBS | d1b_col]
  - DVE: din = d*dw+db; z = Wrep*din + brep (split io/g so the sigmoid
    starts earlier); ACT: one Sigmoid on (BS,256) covers both gates
    (no DMA on the Activation queue, so its function table loads once).
  - PE (off critical path): vrep = (d2w x4).T @ d1w; krep accumulates
    d1b.d2w + d2b via a ones-row matmul (ones baked into packM).
  - finale: mul + reduce + scalar-add (tensor_tensor_reduce faults the
    exec unit on HW; stick to plain DVE ops).
"""

import numpy as np

import concourse.bacc as bacc
import concourse.bass as bass
import concourse.mybir as mybir
import concourse.tile as tile
from concourse import bass_utils

N_CORES = 8
B, T, H, L = 32, 100, 128, 64
BS = B // N_CORES  # batch rows per core

F32 = mybir.dt.float32
AF = mybir.ActivationFunctionType
ALU = mybir.AluOpType

PM_COLS = 6 * H + 8   # [W(384) | b(384) | d dw db d2b | 1 1 1 1]
PB_COLS = H + BS + 1  # [d1w (H) | d2w_col xBS | d1b_col]

_BUILD_CACHE = {}


def _build_nc():
    nc = bacc.Bacc("TRN2", target_bir_lowering=False, debug=False)

    packM = nc.dram_tensor("packM", (BS, PM_COLS), F32, kind="ExternalInput")
    packB = nc.dram_tensor("packB", (H, PB_COLS), F32, kind="ExternalInput")
    out = nc.dram_tensor("out", (BS, 1), F32, kind="ExternalOutput")

    W0, B0, X0 = 0, 3 * H, 6 * H  # pack section offsets

    with tile.TileContext(nc) as tc:
        with (
            tc.tile_pool(name="sb", bufs=1) as sb,
            tc.tile_pool(name="ps", bufs=1, space=bass.MemorySpace.PSUM) as ps,
        ):
            pm = sb.tile([BS, PM_COLS], F32)
            pb = sb.tile([H, PB_COLS], F32)
            nc.sync.dma_start(pm[:, :], packM[:, :])
            nc.sync.dma_start(pb[:, :], packB[:, :])

            # din = d * dec_w00 + dec_b0            (BS,1)
            din = sb.tile([BS, 1], F32)
            nc.vector.tensor_scalar(
                din[:, :], pm[:, X0:X0 + 1],
                pm[:, X0 + 1:X0 + 2], pm[:, X0 + 2:X0 + 3],
                ALU.mult, ALU.add,
            )
            # z = Wrep * din + brep, gates [i|o|g]; io first so ACT starts early
            z = sb.tile([BS, 3 * H], F32)
            nc.vector.scalar_tensor_tensor(
                z[:, 0:2 * H], pm[:, W0:W0 + 2 * H], din[:, :],
                pm[:, B0:B0 + 2 * H], ALU.mult, ALU.add,
            )
            nc.vector.scalar_tensor_tensor(
                z[:, 2 * H:3 * H], pm[:, W0 + 2 * H:W0 + 3 * H], din[:, :],
                pm[:, B0 + 2 * H:B0 + 3 * H], ALU.mult, ALU.add,
            )

            sio = sb.tile([BS, 2 * H], F32)  # sigmoid(i) | sigmoid(o)
            tg = sb.tile([BS, H], F32)
            nc.scalar.activation(sio[:, :], z[:, 0:2 * H], AF.Sigmoid)
            nc.scalar.activation(tg[:, :], z[:, 2 * H:3 * H], AF.Tanh)
            cst = sb.tile([BS, H], F32)
            nc.vector.tensor_mul(cst[:, :], sio[:, 0:H], tg[:, :])
            tcs = sb.tile([BS, H], F32)
            nc.scalar.activation(tcs[:, :], cst[:, :], AF.Tanh)
            hst = sb.tile([BS, H], F32)
            nc.vector.tensor_mul(hst[:, :], sio[:, H:2 * H], tcs[:, :])

            # vrep[b,h] = sum_j d2w[j] * d1w[j,h]   (BS, H)
            vrep = ps.tile([BS, H], F32)
            nc.tensor.matmul(
                vrep[:, :], pb[:, H:H + BS], pb[:, 0:H], start=True, stop=True
            )
            # krep[b] = sum_j d2w[j] * d1b[j] + d2b (BS, 1)
            krep = ps.tile([BS, 1], F32)
            nc.tensor.matmul(
                krep[:, :], pb[:, H:H + BS], pb[:, H + BS:H + BS + 1],
                start=True, stop=False,
            )
            nc.tensor.matmul(
                krep[:, :], pm[0:1, X0 + 4:X0 + 8], pm[0:1, X0 + 3:X0 + 4],
                start=False, stop=True,
            )

            # out[b] = sum_h h[b,h]*vrep[b,h] + krep[b]
            scratch = sb.tile([BS, H], F32)
            res = sb.tile([BS, 1], F32)
            nc.vector.tensor_mul(scratch[:, :], hst[:, :], vrep[:, :])
            nc.vector.tensor_reduce(
                res[:, :], scratch[:, :], mybir.AxisListType.X, ALU.add
            )
            nc.vector.tensor_scalar_add(res[:, :], res[:, :], krep[:, 0:1])
            nc.sync.dma_start(out[:, :], res[:, :])

    nc.compile()
    return nc


def get_nc():
    if "nc" not in _BUILD_CACHE:
        _BUILD_CACHE["nc"] = _build_nc()
    return _BUILD_CACHE["nc"]


def make_in_maps(inputs):
    f = lambda k: np.asarray(inputs[k], dtype=np.float32)
    d = f("d")
    wihd = f("W_ih_d").reshape(4 * H)
    b_d = f("b_d").reshape(4 * H)
    dw = f("dec_w").reshape(1, H + 1)[0, 0]
    db = f("dec_b").reshape(1)[0]
    d1w = f("d1_w").reshape(H, 2 * H)
    d1b = f("d1_b").reshape(H)
    d2w = f("d2_w").reshape(H)
    d2b = f("d2_b").reshape(1)[0]

    X0 = 6 * H
    base = np.empty(PM_COLS, np.float32)  # batch-independent part
    base[0:H] = wihd[0:H]                  # W_i
    base[H:2 * H] = wihd[3 * H:4 * H]      # W_o
    base[2 * H:3 * H] = wihd[2 * H:3 * H]  # W_g
    base[3 * H:4 * H] = b_d[0:H]
    base[4 * H:5 * H] = b_d[3 * H:4 * H]
    base[5 * H:6 * H] = b_d[2 * H:3 * H]
    base[X0 + 1] = dw
    base[X0 + 2] = db
    base[X0 + 3] = d2b
    base[X0 + 4:X0 + 8] = 1.0

    packB = np.empty((H, PB_COLS), np.float32)
    packB[:, 0:H] = d1w[:, 0:H]
    packB[:, H:H + BS] = d2w[:, None]
    packB[:, H + BS] = d1b

    in_maps = []
    for c in range(N_CORES):
        packM = np.tile(base, (BS, 1))
        packM[:, X0] = d[c * BS:(c + 1) * BS, T - 2]  # this core's d[:, T-2]
        in_maps.append({"packM": packM, "packB": packB})
    return in_maps


def run_spmd(inputs, trace=False):
    """Returns (full_output (B,), BassKernelResults)."""
    nc = get_nc()
    res = bass_utils.run_bass_kernel_spmd(
        nc, make_in_maps(inputs), list(range(N_CORES)), trace=trace
    )
    outs = [np.asarray(res.results[c]["out"]).reshape(BS) for c in range(N_CORES)]
    full = np.concatenate(outs).astype(np.float32)
    return full, res


def kernel(**inputs) -> np.ndarray:
    full, _ = run_spmd(inputs, trace=False)
    return full


# revision 11
# speedup vs baseline: 1.0118x; 1.0064x over previous
"""Trainium2 Bass kernel for nn_DSA (dual-stage attention RNN).

Mathematical collapse used (exact, not approximate):
  - In the reference scan, beta = log_softmax(sc, axis=-1) over a SIZE-1
    axis, which is identically zero for any finite input.  Hence
    ctx_new = einsum('bt,bth->bh', 0, enc_h) == 0 exactly, so the carried
    context is zero at every step and the decoder input at step t is
    din_t = d[:, t] * dec_w[0,0] + dec_b[0].
  - The carried h_s is never read inside the step, so only the final
    step's h_s (t = T-2) reaches the head.  The encoder LSTM, s1, and the
    whole attention pipeline are dead code w.r.t. the output.
  - feat = [h_s, ctx] with ctx == 0, so the head reduces to
      out[b] = h_s[b,:] @ v + k0,
      v  = d1_w[:, :H].T @ d2_w[0,:],     k0 = d1_b @ d2_w[0,:] + d2_b[0]
  where h_s = sigmoid(o) * tanh(sigmoid(i) * tanh(g)) and
  [i,f,g,o] = din * W_ih_d[:,0] + b_d  (f unused since c0 == 0).

Sharding: pure data parallel over batch (B=32 -> 4 rows per core x 8).
All weights replicated; each core computes its 4 outputs independently.
Host-side work is layout only (slicing / replication / concatenation);
every arithmetic op ((d*dw+db), the LSTM cell, v, k0, h@v+k0) runs on
device.

Device schedule (per core, BS=4, batch on partitions):
  - TWO input DMAs on separate queues (sync HWDGE + gpsimd SWDGE):
      packM (BS, 776): [W_i|W_o|W_g | b_i|b_o|b_g | d_col dw db d2b 1x4]
      packB (H, 133):  [d1_w[:, :H] | d2w_col xBS | d1b_col]
  - DVE: din = d*dw+db; z = Wrep*din + brep (split io/g so the sigmoid
    starts earlier); ACT: one Sigmoid on (BS,256) covers both gates
    (no DMA on the Activation queue, so its function table loads once).
  - PE (off critical path): vrep = (d2w x4).T @ d1w; krep accumulates
    d1b.d2w + d2b via a ones-row matmul (ones baked into packM).
  - finale: mul + reduce + scalar-add (tensor_tensor_reduce faults the
    exec unit on HW; stick to plain DVE ops).
"""

import numpy as np

import concourse.bacc as bacc
import concourse.bass as bass
import concourse.mybir as mybir
import concourse.tile as tile
from concourse import bass_utils

N_CORES = 8
B, T, H, L = 32, 100, 128, 64
BS = B // N_CORES  # batch rows per core

F32 = mybir.dt.float32
AF = mybir.ActivationFunctionType
ALU = mybir.AluOpType

PM_COLS = 6 * H + 8   # [W(384) | b(384) | d dw db d2b | 1 1 1 1]
PB_COLS = H + BS + 1  # [d1w (H) | d2w_col xBS | d1b_col]

_BUILD_CACHE = {}


def _build_nc():
    nc = bacc.Bacc("TRN2", target_bir_lowering=False, debug=False)

    packM = nc.dram_tensor("packM", (BS, PM_COLS), F32, kind="ExternalInput")
    packB = nc.dram_tensor("packB", (H, PB_COLS), F32, kind="ExternalInput")
    out = nc.dram_tensor("out", (BS, 1), F32, kind="ExternalOutput")

    W0, B0, X0 = 0, 3 * H, 6 * H  # pack section offsets

    with tile.TileContext(nc) as tc:
        with (
            tc.tile_pool(name="sb", bufs=1) as sb,
            tc.tile_pool(name="ps", bufs=1, space=bass.MemorySpace.PSUM) as ps,
        ):
            pm = sb.tile([BS, PM_COLS], F32)
            pb = sb.tile([H, PB_COLS], F32)
            nc.sync.dma_start(pm[:, :], packM[:, :])
            nc.gpsimd.dma_start(pb[:, :], packB[:, :])

            # din = d * dec_w00 + dec_b0            (BS,1)
            din = sb.tile([BS, 1], F32)
            nc.vector.tensor_scalar(
                din[:, :], pm[:, X0:X0 + 1],
                pm[:, X0 + 1:X0 + 2], pm[:, X0 + 2:X0 + 3],
                ALU.mult, ALU.add,
            )
            # z = Wrep * din + brep, gates [i|o|g]; io first so ACT starts early
            z = sb.tile([BS, 3 * H], F32)
            nc.vector.scalar_tensor_tensor(
                z[:, 0:2 * H], pm[:, W0:W0 + 2 * H], din[:, :],
                pm[:, B0:B0 + 2 * H], ALU.mult, ALU.add,
            )
            nc.vector.scalar_tensor_tensor(
                z[:, 2 * H:3 * H], pm[:, W0 + 2 * H:W0 + 3 * H], din[:, :],
                pm[:, B0 + 2 * H:B0 + 3 * H], ALU.mult, ALU.add,
            )

            sio = sb.tile([BS, 2 * H], F32)  # sigmoid(i) | sigmoid(o)
            tg = sb.tile([BS, H], F32)
            nc.scalar.activation(sio[:, :], z[:, 0:2 * H], AF.Sigmoid)
            nc.scalar.activation(tg[:, :], z[:, 2 * H:3 * H], AF.Tanh)
            cst = sb.tile([BS, H], F32)
            nc.vector.tensor_mul(cst[:, :], sio[:, 0:H], tg[:, :])
            tcs = sb.tile([BS, H], F32)
            nc.scalar.activation(tcs[:, :], cst[:, :], AF.Tanh)
            hst = sb.tile([BS, H], F32)
            nc.vector.tensor_mul(hst[:, :], sio[:, H:2 * H], tcs[:, :])

            # vrep[b,h] = sum_j d2w[j] * d1w[j,h]   (BS, H)
            vrep = ps.tile([BS, H], F32)
            nc.tensor.matmul(
                vrep[:, :], pb[:, H:H + BS], pb[:, 0:H], start=True, stop=True
            )
            # krep[b] = sum_j d2w[j] * d1b[j] + d2b (BS, 1)
            krep = ps.tile([BS, 1], F32)
            nc.tensor.matmul(
                krep[:, :], pb[:, H:H + BS], pb[:, H + BS:H + BS + 1],
                start=True, stop=False,
            )
            nc.tensor.matmul(
                krep[:, :], pm[0:1, X0 + 4:X0 + 8], pm[0:1, X0 + 3:X0 + 4],
                start=False, stop=True,
            )

            # out[b] = sum_h h[b,h]*vrep[b,h] + krep[b]
            scratch = sb.tile([BS, H], F32)
            res = sb.tile([BS, 1], F32)
            nc.vector.tensor_mul(scratch[:, :], hst[:, :], vrep[:, :])
            nc.vector.tensor_reduce(
                res[:, :], scratch[:, :], mybir.AxisListType.X, ALU.add
            )
            nc.vector.tensor_scalar_add(res[:, :], res[:, :], krep[:, 0:1])
            nc.sync.dma_start(out[:, :], res[:, :])

    nc.compile()
    return nc


def get_nc():
    if "nc" not in _BUILD_CACHE:
        _BUILD_CACHE["nc"] = _build_nc()
    return _BUILD_CACHE["nc"]


def make_in_maps(inputs):
    f = lambda k: np.asarray(inputs[k], dtype=np.float32)
    d = f("d")
    wihd = f("W_ih_d").reshape(4 * H)
    b_d = f("b_d").reshape(4 * H)
    dw = f("dec_w").reshape(1, H + 1)[0, 0]
    db = f("dec_b").reshape(1)[0]
    d1w = f("d1_w").reshape(H, 2 * H)
    d1b = f("d1_b").reshape(H)
    d2w = f("d2_w").reshape(H)
    d2b = f("d2_b").reshape(1)[0]

    X0 = 6 * H
    base = np.empty(PM_COLS, np.float32)  # batch-independent part
    base[0:H] = wihd[0:H]                  # W_i
    base[H:2 * H] = wihd[3 * H:4 * H]      # W_o
    base[2 * H:3 * H] = wihd[2 * H:3 * H]  # W_g
    base[3 * H:4 * H] = b_d[0:H]
    base[4 * H:5 * H] = b_d[3 * H:4 * H]
    base[5 * H:6 * H] = b_d[2 * H:3 * H]
    base[X0 + 1] = dw
    base[X0 + 2] = db
    base[X0 + 3] = d2b
    base[X0 + 4:X0 + 8] = 1.0

    packB = np.empty((H, PB_COLS), np.float32)
    packB[:, 0:H] = d1w[:, 0:H]
    packB[:, H:H + BS] = d2w[:, None]
    packB[:, H + BS] = d1b

    in_maps = []
    for c in range(N_CORES):
        packM = np.tile(base, (BS, 1))
        packM[:, X0] = d[c * BS:(c + 1) * BS, T - 2]  # this core's d[:, T-2]
        in_maps.append({"packM": packM, "packB": packB})
    return in_maps


def run_spmd(inputs, trace=False):
    """Returns (full_output (B,), BassKernelResults)."""
    nc = get_nc()
    res = bass_utils.run_bass_kernel_spmd(
        nc, make_in_maps(inputs), list(range(N_CORES)), trace=trace
    )
    outs = [np.asarray(res.results[c]["out"]).reshape(BS) for c in range(N_CORES)]
    full = np.concatenate(outs).astype(np.float32)
    return full, res


def kernel(**inputs) -> np.ndarray:
    full, _ = run_spmd(inputs, trace=False)
    return full


# revision 12
# speedup vs baseline: 1.0249x; 1.0129x over previous
"""Trainium2 Bass kernel for nn_DSA (dual-stage attention RNN).

Mathematical collapse used (exact, not approximate):
  - In the reference scan, beta = log_softmax(sc, axis=-1) over a SIZE-1
    axis, which is identically zero for any finite input.  Hence
    ctx_new = einsum('bt,bth->bh', 0, enc_h) == 0 exactly, so the carried
    context is zero at every step and the decoder input at step t is
    din_t = d[:, t] * dec_w[0,0] + dec_b[0].
  - The carried h_s is never read inside the step, so only the final
    step's h_s (t = T-2) reaches the head.  The encoder LSTM, s1, and the
    whole attention pipeline are dead code w.r.t. the output.
  - feat = [h_s, ctx] with ctx == 0, so the head reduces to
      out[b] = h_s[b,:] @ v + k0,
      v  = d1_w[:, :H].T @ d2_w[0,:],     k0 = d1_b @ d2_w[0,:] + d2_b[0]
  where h_s = sigmoid(o) * tanh(sigmoid(i) * tanh(g)) and
  [i,f,g,o] = din * W_ih_d[:,0] + b_d  (f unused since c0 == 0).

Sharding: pure data parallel over batch (B=32 -> 4 rows per core x 8).
All weights replicated; each core computes its 4 outputs independently.
Host-side work is layout only (slicing / replication / concatenation);
every arithmetic op ((d*dw+db), the LSTM cell, v, k0, h@v+k0) runs on
device.

Device schedule (per core, BS=4, batch on partitions):
  - TWO input DMAs on separate queues (sync HWDGE + gpsimd SWDGE):
      packM (BS, 776): [W_i|W_o|W_g | b_i|b_o|b_g | d_col dw db d2b 1x4]
      packB (H, 133):  [d1_w[:, :H] | d2w_col xBS | d1b_col]
  - DVE: din = d*dw+db; z = Wrep*din + brep (split io/g so the sigmoid
    starts earlier); ACT: one Sigmoid on (BS,256) covers both gates
    (no DMA on the Activation queue, so its function table loads once).
  - PE (off critical path): vrep = (d2w x4).T @ d1w; krep accumulates
    d1b.d2w + d2b via a ones-row matmul (ones baked into packM).
  - finale: mul + reduce + scalar-add (tensor_tensor_reduce faults the
    exec unit on HW; stick to plain DVE ops).
"""

import numpy as np

import concourse.bacc as bacc
import concourse.bass as bass
import concourse.mybir as mybir
import concourse.tile as tile
from concourse import bass_utils

N_CORES = 8
B, T, H, L = 32, 100, 128, 64
BS = B // N_CORES  # batch rows per core

F32 = mybir.dt.float32
AF = mybir.ActivationFunctionType
ALU = mybir.AluOpType

PM_COLS = 6 * H + 8   # [W(384) | b(384) | d dw db d2b | 1 1 1 1]
PB_COLS = H + BS + 1  # [d1w (H) | d2w_col xBS | d1b_col]

_BUILD_CACHE = {}


def _build_nc():
    nc = bacc.Bacc("TRN2", target_bir_lowering=False, debug=False)

    packM = nc.dram_tensor("packM", (BS, PM_COLS), F32, kind="ExternalInput")
    packB = nc.dram_tensor("packB", (H, PB_COLS), F32, kind="ExternalInput")
    out = nc.dram_tensor("out", (BS, 1), F32, kind="ExternalOutput")

    W0, B0, X0 = 0, 3 * H, 6 * H  # pack section offsets

    with tile.TileContext(nc) as tc:
        with (
            tc.tile_pool(name="sb", bufs=1) as sb,
            tc.tile_pool(name="ps", bufs=1, space=bass.MemorySpace.PSUM) as ps,
        ):
            pm = sb.tile([BS, PM_COLS], F32)
            pb = sb.tile([H, PB_COLS], F32)
            nc.sync.dma_start(pm[:, :], packM[:, :])
            nc.gpsimd.dma_start(pb[:, :], packB[:, :])

            # din = d * dec_w00 + dec_b0            (BS,1)
            din = sb.tile([BS, 1], F32)
            nc.vector.tensor_scalar(
                din[:, :], pm[:, X0:X0 + 1],
                pm[:, X0 + 1:X0 + 2], pm[:, X0 + 2:X0 + 3],
                ALU.mult, ALU.add,
            )
            # z = Wrep * din + brep, gates [i|o|g]; io first so ACT starts early
            z = sb.tile([BS, 3 * H], F32)
            nc.vector.scalar_tensor_tensor(
                z[:, 0:2 * H], pm[:, W0:W0 + 2 * H], din[:, :],
                pm[:, B0:B0 + 2 * H], ALU.mult, ALU.add,
            )
            nc.vector.scalar_tensor_tensor(
                z[:, 2 * H:3 * H], pm[:, W0 + 2 * H:W0 + 3 * H], din[:, :],
                pm[:, B0 + 2 * H:B0 + 3 * H], ALU.mult, ALU.add,
            )

            # vrep[b,h] = sum_j d2w[j] * d1w[j,h]   (BS, H)
            vrep = ps.tile([BS, H], F32)
            nc.tensor.matmul(
                vrep[:, :], pb[:, H:H + BS], pb[:, 0:H], start=True, stop=True
            )
            # krep[b] = sum_j d2w[j] * d1b[j] + d2b (BS, 1)
            krep = ps.tile([BS, 1], F32)
            nc.tensor.matmul(
                krep[:, :], pb[:, H:H + BS], pb[:, H + BS:H + BS + 1],
                start=True, stop=False,
            )
            nc.tensor.matmul(
                krep[:, :], pm[0:1, X0 + 4:X0 + 8], pm[0:1, X0 + 3:X0 + 4],
                start=False, stop=True,
            )

            sio = sb.tile([BS, 2 * H], F32)  # sigmoid(i) | sigmoid(o)
            tg = sb.tile([BS, H], F32)
            nc.scalar.activation(sio[:, :], z[:, 0:2 * H], AF.Sigmoid)
            nc.scalar.activation(tg[:, :], z[:, 2 * H:3 * H], AF.Tanh)
            cst = sb.tile([BS, H], F32)
            nc.vector.tensor_mul(cst[:, :], sio[:, 0:H], tg[:, :])
            # stage krep into the widened scratch column now - the DVE is
            # otherwise idle while tanh(c) runs on ACT, and this lets one
            # widened reduce absorb the +k0 (drops the tail scalar-add)
            scratch = sb.tile([BS, H + 1], F32)
            nc.vector.tensor_copy(scratch[:, H:H + 1], krep[:, 0:1])
            tcs = sb.tile([BS, H], F32)
            nc.scalar.activation(tcs[:, :], cst[:, :], AF.Tanh)
            hst = sb.tile([BS, H], F32)
            nc.vector.tensor_mul(hst[:, :], sio[:, H:2 * H], tcs[:, :])

            # out[b] = sum_h h[b,h]*vrep[b,h] + krep[b] (krep staged above)
            res = sb.tile([BS, 1], F32)
            nc.vector.tensor_mul(scratch[:, 0:H], hst[:, :], vrep[:, :])
            nc.vector.tensor_reduce(
                res[:, :], scratch[:, :], mybir.AxisListType.X, ALU.add
            )
            nc.sync.dma_start(out[:, :], res[:, :])

    nc.compile()
    return nc


def get_nc():
    if "nc" not in _BUILD_CACHE:
        _BUILD_CACHE["nc"] = _build_nc()
    return _BUILD_CACHE["nc"]


def make_in_maps(inputs):
    f = lambda k: np.asarray(inputs[k], dtype=np.float32)
    d = f("d")
    wihd = f("W_ih_d").reshape(4 * H)
    b_d = f("b_d").reshape(4 * H)
    dw = f("dec_w").reshape(1, H + 1)[0, 0]
    db = f("dec_b").reshape(1)[0]
    d1w = f("d1_w").reshape(H, 2 * H)
    d1b = f("d1_b").reshape(H)
    d2w = f("d2_w").reshape(H)
    d2b = f("d2_b").reshape(1)[0]

    X0 = 6 * H
    base = np.empty(PM_COLS, np.float32)  # batch-independent part
    base[0:H] = wihd[0:H]                  # W_i
    base[H:2 * H] = wihd[3 * H:4 * H]      # W_o
    base[2 * H:3 * H] = wihd[2 * H:3 * H]  # W_g
    base[3 * H:4 * H] = b_d[0:H]
    base[4 * H:5 * H] = b_d[3 * H:4 * H]
    base[5 * H:6 * H] = b_d[2 * H:3 * H]
    base[X0 + 1] = dw
    base[X0 + 2] = db
    base[X0 + 3] = d2b
    base[X0 + 4:X0 + 8] = 1.0

    packB = np.empty((H, PB_COLS), np.float32)
    packB[:, 0:H] = d1w[:, 0:H]
    packB[:, H:H + BS] = d2w[:, None]
    packB[:, H + BS] = d1b

    in_maps = []
    for c in range(N_CORES):
        packM = np.tile(base, (BS, 1))
        packM[:, X0] = d[c * BS:(c + 1) * BS, T - 2]  # this core's d[:, T-2]
        in_maps.append({"packM": packM, "packB": packB})
    return in_maps


def run_spmd(inputs, trace=False):
    """Returns (full_output (B,), BassKernelResults)."""
    nc = get_nc()
    res = bass_utils.run_bass_kernel_spmd(
        nc, make_in_maps(inputs), list(range(N_CORES)), trace=trace
    )
    outs = [np.asarray(res.results[c]["out"]).reshape(BS) for c in range(N_CORES)]
    full = np.concatenate(outs).astype(np.float32)
    return full, res


def kernel(**inputs) -> np.ndarray:
    full, _ = run_spmd(inputs, trace=False)
    return full
